# revision 22
# baseline (speedup 1.0000x reference)
import os
import sys
import time

import numpy as np

for _p in ("/opt/trn_rl_repo",):
    if _p not in sys.path:
        sys.path.insert(0, _p)

PATCH = 7
STRIDE = 3
SAMPLE = 64
H_PARAM = 0.5
ORIENT_W = 0.5
OCC_W = 0.05
EPS_NORM = 1e-05
GRID = 126  # (384 - 7)//3 + 1
S = SAMPLE * SAMPLE  # 4096
NCORES = 8
MSH = S // NCORES  # 512 rows per core
K_REAL = 3136  # 64 * 49

# matmul datapath: "fp8dr" (DoubleRow fp8, pipelined) | "bf16" (simple chunks)
MM_MODE = os.environ.get("KERNEL_MM_MODE", "fp8dr")
FP8_SCALE = 64.0

# fp8 pipelined geometry
KPAD8 = 3328  # 13 * 256
KTP = 13
X_PW = 128  # x piece width (one psum-tile row block)
KX = KTP * 2 * X_PW  # flat free-dim length of one x piece
Y_PIECES = [512, 512, 1024, 1024, 1024]  # sums to S; small lead-in pieces
K_SPLIT0 = 6  # leading k-pairs of piece 0 shipped as a separate first DMA

# bf16 geometry
KPAD16 = 3200
KT16 = 25
CHUNK16 = 1024

LAST_EXEC_NS = None
LAST_RESULT = None
DEVICE_OK = False
TIMES = {}


def _grid_idx(field):
    gx = field[..., 0].reshape(-1)
    gy = field[..., 1].reshape(-1)
    ix = np.clip(np.round((gx + 1.0) * GRID / 2.0 - 0.5).astype(np.int64), 0, GRID - 1)
    iy = np.clip(np.round((gy + 1.0) * GRID / 2.0 - 0.5).astype(np.int64), 0, GRID - 1)
    return iy, ix


def _gather_patches(feat, iy, ix):
    # feat [C, H, W] -> [C*49, S] with torch-unfold channel ordering (c*49 + ki*7+kj)
    C = feat.shape[0]
    n = iy.shape[0]
    by = iy * STRIDE
    bx = ix * STRIDE
    out = np.empty((C, PATCH * PATCH, n), dtype=np.float32)
    for ki in range(PATCH):
        for kj in range(PATCH):
            out[:, ki * PATCH + kj, :] = feat[:, by + ki, bx + kj]
    return out.reshape(C * PATCH * PATCH, n)


def _build_cos_fp8():
    from concourse import bacc, mybir
    from concourse.tile import TileContext

    dt8 = mybir.dt.float8e4
    nc = bacc.Bacc()
    x_ext = nc.declare_dram_parameter("x", [128, 4 * KX], dt8, isOutput=False)
    y_ext = nc.declare_dram_parameter("y", [128, 2 * KTP * S], dt8, isOutput=False)
    out_ext = nc.declare_dram_parameter("cos", [MSH, S], mybir.dt.float32, isOutput=True)

    with TileContext(nc) as tc:
        with tc.tile_pool(name="xp", bufs=1) as xp, \
             tc.tile_pool(name="yp", bufs=3) as yp, \
             tc.tile_pool(name="op", bufs=4) as op, \
             tc.tile_pool(name="pp", bufs=4, space="PSUM") as pp, \
             tc.tile_pool(name="wp", bufs=1, space="PSUM") as wp:
            # HAM warm-up: dummy matmuls keep the PE busy during the DMA
            # lead-in so real matmuls start at the 2.4 GHz clock state.
            dmy = xp.tile([128, 2, 512], dt8, name="dmy")
            nc.vector.memset(dmy, 0)
            wps = wp.tile([128, 512], mybir.dt.float32, name="wps")
            for _ in range(30):
                nc.tensor.matmul(
                    out=wps, lhsT=dmy[:, :, 0:128], rhs=dmy[:, :, :],
                    start=True, stop=True,
                    perf_mode=mybir.MatmulPerfMode.DoubleRow,
                )
            # input DMA issue order on the SP ring sets HW FIFO order:
            # x_m0, then y piece 0 in two k-halves (matmuls start after the
            # first half lands), then remaining x pieces, then later y pieces.
            x_tiles = [None] * 4
            x_tiles[0] = xp.tile([128, KTP, 2, X_PW], dt8, name="x_m0")
            nc.sync.dma_start(out=x_tiles[0], in_=x_ext[:, 0:KX])

            offs = np.cumsum([0] + Y_PIECES[:-1]).tolist()
            w0 = Y_PIECES[0]
            KA = K_SPLIT0
            y0a = xp.tile([128, KA, 2, w0], dt8, name="y0a")
            nc.sync.dma_start(out=y0a, in_=y_ext[:, 0:2 * KA * w0])
            y0b = xp.tile([128, KTP - KA, 2, w0], dt8, name="y0b")
            nc.sync.dma_start(
                out=y0b, in_=y_ext[:, 2 * KA * w0:2 * KTP * w0])

            for m in (1, 2, 3):
                x_tiles[m] = xp.tile([128, KTP, 2, X_PW], dt8, name=f"x_m{m}")
                nc.sync.dma_start(out=x_tiles[m], in_=x_ext[:, m * KX:(m + 1) * KX])

            def rhs_p0(k, c0):
                if k < KA:
                    return y0a[:, k, :, c0:c0 + 512]
                return y0b[:, k - KA, :, c0:c0 + 512]

            cur = (rhs_p0, offs[0], w0)
            def mk_rhs(yt):
                return lambda k, c0: yt[:, k, :, c0:c0 + 512]

            for j in range(len(Y_PIECES)):
                if j + 1 < len(Y_PIECES):
                    wn = Y_PIECES[j + 1]
                    on = offs[j + 1]
                    ytn = yp.tile([128, KTP, 2, wn], dt8, name=f"ypiece_{wn}")
                    nc.sync.dma_start(
                        out=ytn,
                        in_=y_ext[:, 2 * KTP * on:2 * KTP * (on + wn)],
                    )
                    nxt = (mk_rhs(ytn), on, wn)
                else:
                    nxt = None
                rhs_f, off, w = cur
                for m in range(4):
                    for c0 in range(0, w, 512):
                        ps = pp.tile([128, 512], mybir.dt.float32)
                        for k in range(KTP):
                            nc.tensor.matmul(
                                out=ps,
                                lhsT=x_tiles[m][:, k, :, :],
                                rhs=rhs_f(k, c0),
                                start=(k == 0),
                                stop=(k == KTP - 1),
                                perf_mode=mybir.MatmulPerfMode.DoubleRow,
                            )
                        ot = op.tile([128, 512], mybir.dt.float32)
                        nc.vector.tensor_copy(out=ot, in_=ps)
                        nc.scalar.dma_start(
                            out=out_ext[m * 128:(m + 1) * 128,
                                        off + c0:off + c0 + 512],
                            in_=ot,
                        )
                cur = nxt
    nc.finalize()
    return nc


def _build_cos_bf16():
    from concourse import bacc, mybir
    from concourse.tile import TileContext

    sb_dt = mybir.dt.bfloat16
    kt = KT16
    chunk = CHUNK16
    nc = bacc.Bacc()
    x_ext = nc.declare_dram_parameter("x", [KPAD16, MSH], sb_dt, isOutput=False)
    y_ext = nc.declare_dram_parameter("y", [KPAD16, S], sb_dt, isOutput=False)
    out_ext = nc.declare_dram_parameter("cos", [MSH, S], mybir.dt.float32, isOutput=True)
    nch = S // chunk

    with TileContext(nc) as tc:
        with tc.tile_pool(name="xp", bufs=1) as xp, \
             tc.tile_pool(name="yp", bufs=2) as yp, \
             tc.tile_pool(name="op", bufs=4) as op, \
             tc.tile_pool(name="pp", bufs=4, space="PSUM") as pp:
            x_sb = xp.tile([128, kt, MSH], sb_dt)
            nc.sync.dma_start(
                out=x_sb, in_=x_ext.rearrange("(kt p) m -> p kt m", p=128)
            )
            for n in range(nch):
                cols = slice(n * chunk, (n + 1) * chunk)
                y_sb = yp.tile([128, kt, chunk], sb_dt)
                nc.sync.dma_start(
                    out=y_sb,
                    in_=y_ext[:, cols].rearrange("(kt p) s -> p kt s", p=128),
                )
                for m in range(MSH // 128):
                    for nsub in range(chunk // 512):
                        ps = pp.tile([128, 512], mybir.dt.float32)
                        for k in range(kt):
                            nc.tensor.matmul(
                                out=ps,
                                lhsT=x_sb[:, k, m * 128:(m + 1) * 128],
                                rhs=y_sb[:, k, nsub * 512:(nsub + 1) * 512],
                                start=(k == 0),
                                stop=(k == kt - 1),
                            )
                        ot = op.tile([128, 512], mybir.dt.float32)
                        nc.vector.tensor_copy(out=ot, in_=ps)
                        nc.sync.dma_start(
                            out=out_ext[m * 128:(m + 1) * 128,
                                        n * chunk + nsub * 512:
                                        n * chunk + (nsub + 1) * 512],
                            in_=ot,
                        )
    nc.finalize()
    return nc


def _pack_fp8_x(xq_shard):
    # xq_shard [KPAD8, MSH] fp8 -> [128, 4*KX] piece-major flat SBUF layout
    a = xq_shard.reshape(KTP, 2, 128, MSH).transpose(2, 0, 1, 3)  # [128,13,2,512]
    pieces = [np.ascontiguousarray(a[:, :, :, m * X_PW:(m + 1) * X_PW]).reshape(128, KX)
              for m in range(4)]
    return np.concatenate(pieces, axis=1)


def _pack_fp8_y(yq):
    # yq [KPAD8, S] fp8 -> [128, 2*KTP*S] piece-major flat SBUF layout;
    # piece 0 is split into two k-blocks matching the kernel's y0a/y0b DMAs
    b = yq.reshape(KTP, 2, 128, S).transpose(2, 0, 1, 3)  # [128,13,2,4096]
    offs = np.cumsum([0] + Y_PIECES[:-1]).tolist()
    parts = []
    for idx, (o, w) in enumerate(zip(offs, Y_PIECES)):
        if idx == 0:
            ka = K_SPLIT0
            parts.append(np.ascontiguousarray(
                b[:, :ka, :, o:o + w]).reshape(128, 2 * ka * w))
            parts.append(np.ascontiguousarray(
                b[:, ka:, :, o:o + w]).reshape(128, 2 * (KTP - ka) * w))
        else:
            parts.append(np.ascontiguousarray(
                b[:, :, :, o:o + w]).reshape(128, 2 * KTP * w))
    return np.concatenate(parts, axis=1)


def _device_cos(xn, yn):
    """cos = xn.T @ yn ([K_REAL, S] each), row-sharded over 8 neuroncores."""
    import ml_dtypes
    from concourse.bass_utils import run_bass_kernel_spmd

    mode = MM_MODE
    if mode == "fp8dr":
        nc = _build_cos_fp8()
        np_dt = ml_dtypes.float8_e4m3
        x_pad = np.zeros((KPAD8, S), np_dt)
        x_pad[:K_REAL] = (xn * FP8_SCALE).astype(np_dt)
        y_pad = np.zeros((KPAD8, S), np_dt)
        y_pad[:K_REAL] = (yn * FP8_SCALE).astype(np_dt)
        y_dev = _pack_fp8_y(y_pad)
        in_maps = [
            {"x": _pack_fp8_x(x_pad[:, c * MSH:(c + 1) * MSH]), "y": y_dev}
            for c in range(NCORES)
        ]
        unscale = 1.0 / (FP8_SCALE * FP8_SCALE)
    else:
        nc = _build_cos_bf16()
        np_dt = ml_dtypes.bfloat16
        x_pad = np.zeros((KPAD16, S), np_dt)
        x_pad[:K_REAL] = xn.astype(np_dt)
        y_pad = np.zeros((KPAD16, S), np_dt)
        y_pad[:K_REAL] = yn.astype(np_dt)
        in_maps = [
            {"x": np.ascontiguousarray(x_pad[:, c * MSH:(c + 1) * MSH]), "y": y_pad}
            for c in range(NCORES)
        ]
        unscale = 1.0

    res = run_bass_kernel_spmd(nc, in_maps, list(range(NCORES)))
    global LAST_EXEC_NS, LAST_RESULT
    LAST_RESULT = res
    LAST_EXEC_NS = getattr(res, "exec_time_ns", None)
    cos = np.concatenate([res.results[c]["cos"] for c in range(NCORES)], axis=0)
    if unscale != 1.0:
        cos *= unscale
    return cos


def kernel(target_features, reference_features, target_orient, refer_orient,
           target_field, refer_field):
    global DEVICE_OK
    t0 = time.time()
    iy_t, ix_t = _grid_idx(np.asarray(target_field[0], dtype=np.float32))
    iy_r, ix_r = _grid_idx(np.asarray(refer_field[0], dtype=np.float32))

    tf = _gather_patches(np.asarray(target_features[0], np.float32), iy_t, ix_t)
    rf = _gather_patches(np.asarray(reference_features[0], np.float32), iy_r, ix_r)
    to = _gather_patches(np.asarray(target_orient[0], np.float32), iy_t, ix_t)
    ro = _gather_patches(np.asarray(refer_orient[0], np.float32), iy_r, ix_r)

    # cosine normalization (y-mean centering per reference)
    y_mean = rf.mean(axis=1, keepdims=True)
    xc = tf - y_mean
    yc = rf - y_mean
    xn = xc / (np.linalg.norm(xc, axis=0, keepdims=True) + EPS_NORM)
    yn = yc / (np.linalg.norm(yc, axis=0, keepdims=True) + EPS_NORM)
    t1 = time.time()

    cos = None
    try:
        cos = _device_cos(xn, yn)
        DEVICE_OK = True
    except Exception as e:  # fall back to host if device path unavailable
        sys.stderr.write(f"device path failed ({type(e).__name__}: {e}); numpy fallback\n")
        cos = xn.T @ yn
    t2 = time.time()
    d_total = np.maximum((1.0 - cos) / 2.0, 0.0).astype(np.float32)

    # orientation term: d_or = (X2[s] + Y2[t] - 2*sum_i |dot_i|) / (2*49)
    xs = to.reshape(2, 49, S)
    ys = ro.reshape(2, 49, S)
    X2 = (xs * xs).sum(axis=0)  # [49, S]
    Y2 = (ys * ys).sum(axis=0)
    A = np.zeros((S, S), np.float32)
    for i in range(49):
        A += np.abs(xs[:, i, :].T @ ys[:, i, :])
    d_or = (X2.sum(0)[:, None] + Y2.sum(0)[None, :] - 2.0 * A) / (2.0 * 49)
    np.maximum(d_or, 0.0, out=d_or)
    d_total += ORIENT_W * d_or

    # occurrence penalty
    min_idx = np.argmin(d_total, axis=1)
    counts = np.bincount(min_idx, minlength=S).astype(np.float32)
    norm_factor = d_total.shape[0] / d_total.shape[1]
    d_total += OCC_W * (counts / norm_factor)[None, :]

    # loss
    min_d = d_total.min(axis=1, keepdims=True)
    rel = d_total / (min_d + 1e-05)
    w = np.exp((1.0 - rel) / H_PARAM)
    nw = w / w.sum(axis=1, keepdims=True)
    loss = -np.log(nw.max(axis=1)).mean()
    t3 = time.time()
    TIMES.update(prep=t1 - t0, device=t2 - t1, post=t3 - t2)
    return np.float32(loss)


# revision 27
# speedup vs baseline: 1.0671x; 1.0671x over previous
import os
import sys
import time

import numpy as np

for _p in ("/opt/trn_rl_repo",):
    if _p not in sys.path:
        sys.path.insert(0, _p)

PATCH = 7
STRIDE = 3
SAMPLE = 64
H_PARAM = 0.5
ORIENT_W = 0.5
OCC_W = 0.05
EPS_NORM = 1e-05
GRID = 126  # (384 - 7)//3 + 1
S = SAMPLE * SAMPLE  # 4096
NCORES = 8
MSH = S // NCORES  # 512 rows per core
K_REAL = 3136  # 64 * 49

# matmul datapath: "fp8dr" (DoubleRow fp8, pipelined) | "bf16" (simple chunks)
MM_MODE = os.environ.get("KERNEL_MM_MODE", "fp8dr")
FP8_SCALE = 64.0

# fp8 pipelined geometry
KPAD8 = 3328  # 13 * 256
KTP = 13
X_PW = 128  # x piece width (one psum-tile row block)
KX = KTP * 2 * X_PW  # flat free-dim length of one x piece
Y_PIECES = [512, 512, 1024, 1024, 1024]  # sums to S; small lead-in pieces
K_SPLIT0 = 6  # leading k-pairs of piece 0 shipped as a separate first DMA

# bf16 geometry
KPAD16 = 3200
KT16 = 25
CHUNK16 = 1024

LAST_EXEC_NS = None
LAST_RESULT = None
DEVICE_OK = False
TIMES = {}


def _grid_idx(field):
    gx = field[..., 0].reshape(-1)
    gy = field[..., 1].reshape(-1)
    ix = np.clip(np.round((gx + 1.0) * GRID / 2.0 - 0.5).astype(np.int64), 0, GRID - 1)
    iy = np.clip(np.round((gy + 1.0) * GRID / 2.0 - 0.5).astype(np.int64), 0, GRID - 1)
    return iy, ix


def _gather_patches(feat, iy, ix):
    # feat [C, H, W] -> [C*49, S] with torch-unfold channel ordering (c*49 + ki*7+kj)
    C = feat.shape[0]
    n = iy.shape[0]
    by = iy * STRIDE
    bx = ix * STRIDE
    out = np.empty((C, PATCH * PATCH, n), dtype=np.float32)
    for ki in range(PATCH):
        for kj in range(PATCH):
            out[:, ki * PATCH + kj, :] = feat[:, by + ki, bx + kj]
    return out.reshape(C * PATCH * PATCH, n)


def _build_cos_fp8():
    from concourse import bacc, mybir
    from concourse.tile import TileContext

    dt8 = mybir.dt.float8e4
    nc = bacc.Bacc()
    x_ext = nc.declare_dram_parameter("x", [128, 4 * KX], dt8, isOutput=False)
    y_ext = nc.declare_dram_parameter("y", [128, 2 * KTP * S], dt8, isOutput=False)
    out_ext = nc.declare_dram_parameter("cos", [MSH, S], mybir.dt.float32, isOutput=True)

    with TileContext(nc) as tc:
        with tc.tile_pool(name="xp", bufs=1) as xp, \
             tc.tile_pool(name="yp", bufs=3) as yp, \
             tc.tile_pool(name="op", bufs=4) as op, \
             tc.tile_pool(name="pp", bufs=4, space="PSUM") as pp:
            # input DMA issue order on the SP ring sets HW FIFO order:
            # x_m0 first, then y piece 0, then remaining x pieces, then the
            # prefetched later y pieces. All are flat contiguous lines.
            x_tiles = [None] * 4
            x_tiles[0] = xp.tile([128, KTP, 2, X_PW], dt8, name="x_m0")
            nc.sync.dma_start(out=x_tiles[0], in_=x_ext[:, 0:KX])

            offs = np.cumsum([0] + Y_PIECES[:-1]).tolist()
            w0 = Y_PIECES[0]
            y0a = xp.tile([128, K_SPLIT0, 2, w0], dt8, name="y0a")
            nc.sync.dma_start(out=y0a, in_=y_ext[:, 0:2 * K_SPLIT0 * w0])
            y0b = xp.tile([128, KTP - K_SPLIT0, 2, w0], dt8, name="y0b")
            nc.sync.dma_start(out=y0b, in_=y_ext[:, 2 * K_SPLIT0 * w0:2 * KTP * w0])

            for m in (1, 2, 3):
                x_tiles[m] = xp.tile([128, KTP, 2, X_PW], dt8, name=f"x_m{m}")
                nc.sync.dma_start(out=x_tiles[m], in_=x_ext[:, m * KX:(m + 1) * KX])

            def rhs_p0(k, c0):
                if k < K_SPLIT0:
                    return y0a[:, k, :, c0:c0 + 512]
                return y0b[:, k - K_SPLIT0, :, c0:c0 + 512]

            def mk_rhs(yt):
                return lambda k, c0: yt[:, k, :, c0:c0 + 512]

            cur = (rhs_p0, offs[0], w0)
            for j in range(len(Y_PIECES)):
                if j + 1 < len(Y_PIECES):
                    wn = Y_PIECES[j + 1]
                    on = offs[j + 1]
                    ytn = yp.tile([128, KTP, 2, wn], dt8, name=f"ypiece_{wn}")
                    nc.sync.dma_start(
                        out=ytn,
                        in_=y_ext[:, 2 * KTP * on:2 * KTP * (on + wn)],
                    )
                    nxt = (mk_rhs(ytn), on, wn)
                else:
                    nxt = None
                rhs_f, off, w = cur
                for m in range(4):
                    for c0 in range(0, w, 512):
                        ps = pp.tile([128, 512], mybir.dt.float32)
                        for k in range(KTP):
                            nc.tensor.matmul(
                                out=ps,
                                lhsT=x_tiles[m][:, k, :, :],
                                rhs=rhs_f(k, c0),
                                start=(k == 0),
                                stop=(k == KTP - 1),
                                perf_mode=mybir.MatmulPerfMode.DoubleRow,
                            )
                        ot = op.tile([128, 512], mybir.dt.float32)
                        nc.vector.tensor_copy(out=ot, in_=ps)
                        nc.scalar.dma_start(
                            out=out_ext[m * 128:(m + 1) * 128,
                                        off + c0:off + c0 + 512],
                            in_=ot,
                        )
                cur = nxt
    nc.finalize()
    return nc


def _build_cos_bf16():
    from concourse import bacc, mybir
    from concourse.tile import TileContext

    sb_dt = mybir.dt.bfloat16
    kt = KT16
    chunk = CHUNK16
    nc = bacc.Bacc()
    x_ext = nc.declare_dram_parameter("x", [KPAD16, MSH], sb_dt, isOutput=False)
    y_ext = nc.declare_dram_parameter("y", [KPAD16, S], sb_dt, isOutput=False)
    out_ext = nc.declare_dram_parameter("cos", [MSH, S], mybir.dt.float32, isOutput=True)
    nch = S // chunk

    with TileContext(nc) as tc:
        with tc.tile_pool(name="xp", bufs=1) as xp, \
             tc.tile_pool(name="yp", bufs=2) as yp, \
             tc.tile_pool(name="op", bufs=4) as op, \
             tc.tile_pool(name="pp", bufs=4, space="PSUM") as pp:
            x_sb = xp.tile([128, kt, MSH], sb_dt)
            nc.sync.dma_start(
                out=x_sb, in_=x_ext.rearrange("(kt p) m -> p kt m", p=128)
            )
            for n in range(nch):
                cols = slice(n * chunk, (n + 1) * chunk)
                y_sb = yp.tile([128, kt, chunk], sb_dt)
                nc.sync.dma_start(
                    out=y_sb,
                    in_=y_ext[:, cols].rearrange("(kt p) s -> p kt s", p=128),
                )
                for m in range(MSH // 128):
                    for nsub in range(chunk // 512):
                        ps = pp.tile([128, 512], mybir.dt.float32)
                        for k in range(kt):
                            nc.tensor.matmul(
                                out=ps,
                                lhsT=x_sb[:, k, m * 128:(m + 1) * 128],
                                rhs=y_sb[:, k, nsub * 512:(nsub + 1) * 512],
                                start=(k == 0),
                                stop=(k == kt - 1),
                            )
                        ot = op.tile([128, 512], mybir.dt.float32)
                        nc.vector.tensor_copy(out=ot, in_=ps)
                        nc.sync.dma_start(
                            out=out_ext[m * 128:(m + 1) * 128,
                                        n * chunk + nsub * 512:
                                        n * chunk + (nsub + 1) * 512],
                            in_=ot,
                        )
    nc.finalize()
    return nc


def _pack_fp8_x(xq_shard):
    # xq_shard [KPAD8, MSH] fp8 -> [128, 4*KX] piece-major flat SBUF layout
    a = xq_shard.reshape(KTP, 2, 128, MSH).transpose(2, 0, 1, 3)  # [128,13,2,512]
    pieces = [np.ascontiguousarray(a[:, :, :, m * X_PW:(m + 1) * X_PW]).reshape(128, KX)
              for m in range(4)]
    return np.concatenate(pieces, axis=1)


def _pack_fp8_y(yq):
    # yq [KPAD8, S] fp8 -> [128, 2*KTP*S] piece-major flat SBUF layout;
    # piece 0 is split into two k-blocks matching the kernel's y0a/y0b DMAs
    b = yq.reshape(KTP, 2, 128, S).transpose(2, 0, 1, 3)  # [128,13,2,4096]
    offs = np.cumsum([0] + Y_PIECES[:-1]).tolist()
    parts = []
    for idx, (o, w) in enumerate(zip(offs, Y_PIECES)):
        if idx == 0:
            ka = K_SPLIT0
            parts.append(np.ascontiguousarray(
                b[:, :ka, :, o:o + w]).reshape(128, 2 * ka * w))
            parts.append(np.ascontiguousarray(
                b[:, ka:, :, o:o + w]).reshape(128, 2 * (KTP - ka) * w))
        else:
            parts.append(np.ascontiguousarray(
                b[:, :, :, o:o + w]).reshape(128, 2 * KTP * w))
    return np.concatenate(parts, axis=1)


def _device_cos(xn, yn):
    """cos = xn.T @ yn ([K_REAL, S] each), row-sharded over 8 neuroncores."""
    import ml_dtypes
    from concourse.bass_utils import run_bass_kernel_spmd

    mode = MM_MODE
    if mode == "fp8dr":
        nc = _build_cos_fp8()
        np_dt = ml_dtypes.float8_e4m3
        x_pad = np.zeros((KPAD8, S), np_dt)
        x_pad[:K_REAL] = (xn * FP8_SCALE).astype(np_dt)
        y_pad = np.zeros((KPAD8, S), np_dt)
        y_pad[:K_REAL] = (yn * FP8_SCALE).astype(np_dt)
        y_dev = _pack_fp8_y(y_pad)
        in_maps = [
            {"x": _pack_fp8_x(x_pad[:, c * MSH:(c + 1) * MSH]), "y": y_dev}
            for c in range(NCORES)
        ]
        unscale = 1.0 / (FP8_SCALE * FP8_SCALE)
    else:
        nc = _build_cos_bf16()
        np_dt = ml_dtypes.bfloat16
        x_pad = np.zeros((KPAD16, S), np_dt)
        x_pad[:K_REAL] = xn.astype(np_dt)
        y_pad = np.zeros((KPAD16, S), np_dt)
        y_pad[:K_REAL] = yn.astype(np_dt)
        in_maps = [
            {"x": np.ascontiguousarray(x_pad[:, c * MSH:(c + 1) * MSH]), "y": y_pad}
            for c in range(NCORES)
        ]
        unscale = 1.0

    res = run_bass_kernel_spmd(nc, in_maps, list(range(NCORES)))
    global LAST_EXEC_NS, LAST_RESULT
    LAST_RESULT = res
    LAST_EXEC_NS = getattr(res, "exec_time_ns", None)
    cos = np.concatenate([res.results[c]["cos"] for c in range(NCORES)], axis=0)
    if unscale != 1.0:
        cos *= unscale
    return cos


def kernel(target_features, reference_features, target_orient, refer_orient,
           target_field, refer_field):
    global DEVICE_OK
    t0 = time.time()
    iy_t, ix_t = _grid_idx(np.asarray(target_field[0], dtype=np.float32))
    iy_r, ix_r = _grid_idx(np.asarray(refer_field[0], dtype=np.float32))

    tf = _gather_patches(np.asarray(target_features[0], np.float32), iy_t, ix_t)
    rf = _gather_patches(np.asarray(reference_features[0], np.float32), iy_r, ix_r)
    to = _gather_patches(np.asarray(target_orient[0], np.float32), iy_t, ix_t)
    ro = _gather_patches(np.asarray(refer_orient[0], np.float32), iy_r, ix_r)

    # cosine normalization (y-mean centering per reference)
    y_mean = rf.mean(axis=1, keepdims=True)
    xc = tf - y_mean
    yc = rf - y_mean
    xn = xc / (np.linalg.norm(xc, axis=0, keepdims=True) + EPS_NORM)
    yn = yc / (np.linalg.norm(yc, axis=0, keepdims=True) + EPS_NORM)
    t1 = time.time()

    cos = None
    try:
        cos = _device_cos(xn, yn)
        DEVICE_OK = True
    except Exception as e:  # fall back to host if device path unavailable
        sys.stderr.write(f"device path failed ({type(e).__name__}: {e}); numpy fallback\n")
        cos = xn.T @ yn
    t2 = time.time()
    d_total = np.maximum((1.0 - cos) / 2.0, 0.0).astype(np.float32)

    # orientation term: d_or = (X2[s] + Y2[t] - 2*sum_i |dot_i|) / (2*49)
    xs = to.reshape(2, 49, S)
    ys = ro.reshape(2, 49, S)
    X2 = (xs * xs).sum(axis=0)  # [49, S]
    Y2 = (ys * ys).sum(axis=0)
    A = np.zeros((S, S), np.float32)
    for i in range(49):
        A += np.abs(xs[:, i, :].T @ ys[:, i, :])
    d_or = (X2.sum(0)[:, None] + Y2.sum(0)[None, :] - 2.0 * A) / (2.0 * 49)
    np.maximum(d_or, 0.0, out=d_or)
    d_total += ORIENT_W * d_or

    # occurrence penalty
    min_idx = np.argmin(d_total, axis=1)
    counts = np.bincount(min_idx, minlength=S).astype(np.float32)
    norm_factor = d_total.shape[0] / d_total.shape[1]
    d_total += OCC_W * (counts / norm_factor)[None, :]

    # loss
    min_d = d_total.min(axis=1, keepdims=True)
    rel = d_total / (min_d + 1e-05)
    w = np.exp((1.0 - rel) / H_PARAM)
    nw = w / w.sum(axis=1, keepdims=True)
    loss = -np.log(nw.max(axis=1)).mean()
    t3 = time.time()
    TIMES.update(prep=t1 - t0, device=t2 - t1, post=t3 - t2)
    return np.float32(loss)
